# revision 14
# baseline (speedup 1.0000x reference)
"""Cross-attention block (LN -> QKV -> full softmax attention -> proj + residual)
as a Bass/Tile kernel for 8 Trainium2 NeuronCores.

Sharding (hardcoded for B=4, H=W=64, C=U=256):
  core c handles batch b = c//2 and query-half h = c%2 (2048 of 4096 query
  positions), with K/V computed from the full 4096-position context of batch b
  (replicated inside the 2-core group). No collectives needed.

Per-core structure (P = 128 partitions, fp32r single-pass matmuls):
  ctxT [C, keys]   : transposed on the host, DMA'd straight into SBUF
  kT = Wk.T @ ctx  : [U, keys], qT = Wq.T @ x_n : [U, queries]
  v_aug            : natural [keys(P), C+2] with two ones columns (the ones
                     columns turn the attention matmul into a softmax-
                     denominator accumulator; width padded even for f32r)
  scoresT          : [keys(P), queries] PSUM -> exp on ACT (no max-sub:
                     scores are O(+-8), fp32 exp is safe)
  attention        : superblocks of 1024 queries = 2 psum blocks of 512;
                     out = v_aug.T @ p accumulates transposed [C | denom, q]
                     so the projection needs no extra transpose; each kT / v
                     weight load serves 2 matmuls so LDWEIGHTS stays hidden
  epilogue         : denom row -> reciprocal -> tiny PE transpose to [q, 1],
                     proj = atT.T @ Wp scaled by 1/denom on the copy-out,
                     + (x_n + bp) residual
"""

import numpy as np

P = 128
C = 256
U = 256
NQ = 2048          # queries per core
NK = 4096          # keys per core
QT = NQ // P       # 16 query tiles
KT = NK // P       # 32 key tiles
IB = 512           # psum block width (queries)
SB = 1024          # superblock: 2 psum blocks share each kT / v weight load
NSB = NQ // SB     # 2
SCALE = float(U) ** -0.5
LN_EPS = 1e-3

_CACHE = {}
LAST_RESULTS = None


def _build_bass():
    import concourse.bass as bass
    import concourse.tile as tile
    from concourse import bacc, mybir
    from concourse.masks import make_identity

    f32 = mybir.dt.float32
    f32r = mybir.dt.float32r
    bf16 = mybir.dt.bfloat16
    AF = mybir.ActivationFunctionType

    nc = bacc.Bacc("TRN2", debug=False, num_devices=8)

    x_d = nc.dram_tensor("x", [NQ, C], f32, kind="ExternalInput").ap()
    # f32r so the straight bit-copy DMA into the f32r SBUF slab is not a cast
    ctxT_d = nc.dram_tensor("ctxT", [C, NK], f32r, kind="ExternalInput").ap()
    w_d = {
        name: nc.dram_tensor(name, [C, U], f32, kind="ExternalInput").ap()
        for name in ("Wq", "Wk", "Wv", "Wp")
    }
    b_d = {
        name: nc.dram_tensor(name, [U], f32, kind="ExternalInput").ap()
        for name in ("bq", "bk", "bv", "bp")
    }
    gamma_d = nc.dram_tensor("gamma", [C], f32, kind="ExternalInput").ap()
    beta_d = nc.dram_tensor("beta", [C], f32, kind="ExternalInput").ap()
    out_d = nc.dram_tensor("out", [NQ, C], f32, kind="ExternalOutput").ap()

    def bcast(ap1d):
        # [N] dram vector -> [P, N] broadcast read (partition step 0)
        return bass.AP(tensor=ap1d.tensor, offset=ap1d.offset, ap=[[0, P], *ap1d.ap])

    with tile.TileContext(nc) as tc:
        from contextlib import ExitStack

        with ExitStack() as es:
            singles = es.enter_context(tc.tile_pool(name="singles", bufs=1))
            psum = es.enter_context(tc.tile_pool(name="psum", bufs=2, space="PSUM"))
            work = es.enter_context(tc.tile_pool(name="work", bufs=4))
            ln = es.enter_context(tc.tile_pool(name="ln", bufs=4))
            p_pool = es.enter_context(tc.tile_pool(name="p_pool", bufs=4))
            inv_pool = es.enter_context(tc.tile_pool(name="inv_pool", bufs=4))
            fin_pool = es.enter_context(tc.tile_pool(name="fin_pool", bufs=4))

            # ---- constants ----
            ident = singles.tile([P, P], f32)
            make_identity(nc, ident)
            eps_t = singles.tile([P, 1], f32)
            nc.vector.memset(eps_t, LN_EPS)
            one11 = singles.tile([1, 1], f32)
            nc.vector.memset(one11, 1.0)

            # DMA order on the sync queue is the startup critical path:
            # Wk + biases first (kT needs them), then ctxT chunks interleaved
            # with the x tiles (LN runs on DVE/ACT while PE does kT/v).
            w_stage = {}
            w_sb = {}

            def dma_w(name):
                t0 = work.tile([P, 2, U], f32, tag="wstage", name=f"sb0_{name}", bufs=2)
                nc.sync.dma_start(out=t0, in_=w_d[name].rearrange("(a p) u -> p a u", p=P))
                w_stage[name] = t0

            def cast_w(name):
                # round to f32r once so the PE single-pass matmul can consume it
                t = singles.tile([P, 2, U], f32r, name=f"sb_{name}")
                nc.vector.tensor_copy(out=t, in_=w_stage[name])
                w_sb[name] = t

            dma_w("Wk")
            bk_t = singles.tile([P, 2], f32)
            nc.sync.dma_start(out=bk_t, in_=b_d["bk"].rearrange("(a p) -> p a", p=P))
            bq_t = singles.tile([P, 2], f32)
            nc.sync.dma_start(out=bq_t, in_=b_d["bq"].rearrange("(a p) -> p a", p=P))
            cast_w("Wk")

            bv_b = singles.tile([P, C], f32)
            nc.gpsimd.dma_start(out=bv_b, in_=bcast(b_d["bv"]))
            bp_b = singles.tile([P, C], f32)
            nc.gpsimd.dma_start(out=bp_b, in_=bcast(b_d["bp"]))
            gamma_b = singles.tile([P, C], f32)
            nc.gpsimd.dma_start(out=gamma_b, in_=bcast(gamma_d))
            beta_b = singles.tile([P, C], f32)
            nc.gpsimd.dma_start(out=beta_b, in_=bcast(beta_d))

            # ---- persistent slabs ----
            xn = singles.tile([P, QT, C], f32)         # x_n natural (+bp later)
            xnT = singles.tile([P, 2, NQ], f32r)       # x_n transposed [C, rows]
            ctxT = singles.tile([P, 2, NK], f32r)      # context transposed [C, keys]
            kT = singles.tile([P, 2, NK], bf16)        # k transposed [U, keys]
            qT = singles.tile([P, 2, NQ], bf16)        # q transposed [U, queries]
            v_aug = singles.tile([P, KT, C + 2], bf16)  # v natural + ones cols
            atT = singles.tile([P, 2, NQ], f32r)       # attn-out unnormalized [C, q]

            # ctxT arrives pre-transposed from the host; chunked DMA (with the
            # x tiles interleaved) so kT matmuls start on chunk 0 while the
            # rest streams in and LN has inputs early.
            ctxT_src = ctxT_d.rearrange("(a p) j -> p a j", p=P)
            x_t3 = x_d.rearrange("(t p) c -> t p c", p=P)
            x_tiles = []
            NCH = 4
            CHW = NK // NCH
            for ch in range(NCH):
                nc.sync.dma_start(
                    out=ctxT[:, :, ch * CHW:(ch + 1) * CHW],
                    in_=ctxT_src[:, :, ch * CHW:(ch + 1) * CHW],
                )
                for t in range(ch * 4, ch * 4 + 4):
                    x_t = work.tile([P, C], f32, tag="x", name=f"x_{t}", bufs=8)
                    nc.sync.dma_start(out=x_t, in_=x_t3[t])
                    x_tiles.append(x_t)

            # ---- layernorm(x) compute (DVE/ACT; overlaps the PE prep) ----
            for t in range(QT):
                x_t = x_tiles[t]
                st = ln.tile([P, 6], f32, tag="st")
                nc.vector.bn_stats(out=st, in_=x_t)
                mv = ln.tile([P, 2], f32, tag="mv")
                nc.vector.bn_aggr(out=mv, in_=st)
                rstd = ln.tile([P, 1], f32, tag="rstd")
                nc.scalar.activation(out=rstd, in_=mv[:, 1:2], func=AF.Sqrt, bias=eps_t)
                nc.vector.reciprocal(rstd, rstd)
                nmr = ln.tile([P, 1], f32, tag="nmr")
                nc.vector.tensor_mul(nmr, mv[:, 0:1], rstd)
                nc.vector.tensor_scalar_mul(nmr, nmr, -1.0)
                # x_n = x * rstd - mu * rstd, then gamma/beta
                nc.scalar.activation(
                    out=xn[:, t, :], in_=x_t, func=AF.Identity, bias=nmr, scale=rstd
                )
                nc.vector.tensor_mul(xn[:, t, :], xn[:, t, :], gamma_b)
                nc.vector.tensor_add(xn[:, t, :], xn[:, t, :], beta_b)

            # ---- kT[u, j] = sum_c Wk[c, u] * ctx[j, c] (first PE work) ----
            for n in range(NK // 512):
                for b2 in range(2):
                    ps = psum.tile([P, 512], f32, tag="o", bufs=6, name="ps_k")
                    for a in range(2):
                        nc.tensor.matmul(
                            ps,
                            lhsT=w_sb["Wk"][:, a, b2 * P:(b2 + 1) * P],
                            rhs=ctxT[:, a, n * 512:(n + 1) * 512],
                            start=(a == 0),
                            stop=(a == 1),
                        )
                    nc.scalar.activation(
                        out=kT[:, b2, n * 512:(n + 1) * 512],
                        in_=ps,
                        func=AF.Identity,
                        bias=bk_t[:, b2:b2 + 1],
                    )

            dma_w("Wv")
            dma_w("Wq")
            dma_w("Wp")
            cast_w("Wv")

            # ---- v natural [j, c] (+ ones columns for the denominator) ----
            for t in range(KT):
                ps = psum.tile([P, C], f32, tag="o", bufs=6, name="ps_v")
                for a in range(2):
                    nc.tensor.matmul(
                        ps,
                        lhsT=ctxT[:, a, t * P:(t + 1) * P],
                        rhs=w_sb["Wv"][:, a, :],
                        start=(a == 0),
                        stop=(a == 1),
                    )
                nc.vector.tensor_add(v_aug[:, t, 0:C], ps, bv_b)
                nc.scalar.activation(
                    out=v_aug[:, t, C:C + 2], in_=gamma_b[:, 0:2],
                    func=AF.Copy, scale=0.0, bias=1.0,
                )

            cast_w("Wq")
            cast_w("Wp")

            # ---- transpose x_n (PE; LN is long done by now) ----
            for t in range(QT):
                for a in range(2):
                    pt = psum.tile([P, P], f32, tag="s", name="pt_xn")
                    nc.tensor.transpose(pt, xn[:, t, a * P:(a + 1) * P], ident)
                    nc.vector.tensor_copy(out=xnT[:, a, t * P:(t + 1) * P], in_=pt)
                # residual base = x_n + bp (after the transpose reads x_n)
                nc.vector.tensor_add(xn[:, t, :], xn[:, t, :], bp_b)

            # ---- qT[u, i] = sum_c Wq[c, u] * x_n[i, c] ----
            for n in range(NQ // 512):
                for b2 in range(2):
                    ps = psum.tile([P, 512], f32, tag="o", bufs=6, name="ps_q")
                    for a in range(2):
                        nc.tensor.matmul(
                            ps,
                            lhsT=w_sb["Wq"][:, a, b2 * P:(b2 + 1) * P],
                            rhs=xnT[:, a, n * 512:(n + 1) * 512],
                            start=(a == 0),
                            stop=(a == 1),
                        )
                    nc.scalar.activation(
                        out=qT[:, b2, n * 512:(n + 1) * 512],
                        in_=ps,
                        func=AF.Identity,
                        bias=bq_t[:, b2:b2 + 1],
                    )

            # ---- attention: per superblock of 1024 queries ----
            # po[blk] = [atT chunk0 [128,512], atT chunk1 [128,512],
            #            denom rows [2,512]]  accumulated over all 32 key tiles
            VCH = ((0, P), (P, 2 * P), (2 * P, C + 2))

            def run_superblock(sb):
                po = [
                    [
                        psum.tile(
                            [hi - lo, IB], f32, tag="o", bufs=6,
                            name=f"po{sb}_{blk}_{ci}",
                        )
                        for ci, (lo, hi) in enumerate(VCH)
                    ]
                    for blk in range(2)
                ]

                def emit_attn(pts, j):
                    for ci, (lo, hi) in enumerate(VCH):
                        for blk in range(2):
                            nc.tensor.matmul(
                                po[blk][ci],
                                lhsT=v_aug[:, j, lo:hi],
                                rhs=pts[blk],
                                start=(j == 0),
                                stop=(j == KT - 1),
                            )

                pend = None
                for j in range(KT):
                    pss = []
                    for blk in range(2):
                        ps = psum.tile([P, IB], f32, tag="s", name="ps_s")
                        pss.append(ps)
                    for a in range(2):
                        for blk in range(2):
                            nc.tensor.matmul(
                                pss[blk],
                                lhsT=kT[:, a, j * P:(j + 1) * P],
                                rhs=qT[:, a, sb * SB + blk * IB:sb * SB + (blk + 1) * IB],
                                start=(a == 0),
                                stop=(a == 1),
                            )
                    pts = []
                    for blk in range(2):
                        pt = p_pool.tile([P, IB], bf16, tag="p", name="p_exp")
                        nc.scalar.activation(out=pt, in_=pss[blk], func=AF.Exp, scale=SCALE)
                        pts.append(pt)
                    # 1-deep software pipeline: previous j's attention matmuls
                    # are emitted after this j's score matmuls
                    if pend is not None:
                        emit_attn(*pend)
                    pend = (pts, j)
                emit_attn(*pend)
                return po

            def early_epilogue(sb, po):
                # free the 6 po psum banks: copy atT chunks out, build 1/denom
                invs = []
                for blk in range(2):
                    qlo = sb * SB + blk * IB
                    for ci in range(2):
                        nc.vector.tensor_copy(
                            out=atT[:, ci, qlo:qlo + IB], in_=po[blk][ci]
                        )
                    inv_row = inv_pool.tile([1, IB], f32, tag="invrow")
                    nc.vector.reciprocal(inv_row, po[blk][2][0:1, :])
                    invs.append(inv_row)
                return invs

            def late_epilogue(sb, invs):
                for blk in range(2):
                    inv_row = invs[blk]
                    for s in range(IB // P):
                        t = (sb * SB + blk * IB) // P + s
                        # transpose inv_row chunk [1,128] -> [128,1] via PE
                        ps_i = psum.tile([P, 1], f32, tag="s", name="ps_i")
                        nc.tensor.matmul(
                            ps_i,
                            lhsT=inv_row[0:1, s * P:(s + 1) * P],
                            rhs=one11,
                            start=True,
                            stop=True,
                        )
                        inv_t = inv_pool.tile([P, 1], f32, tag="invt", bufs=8)
                        nc.vector.tensor_copy(out=inv_t, in_=ps_i)
                        # proj, scaled by 1/denom on copy-out, + residual
                        ps_p = psum.tile([P, C], f32, tag="s", name="ps_p")
                        for a in range(2):
                            nc.tensor.matmul(
                                ps_p,
                                lhsT=atT[:, a, t * P:(t + 1) * P],
                                rhs=w_sb["Wp"][:, a, :],
                                start=(a == 0),
                                stop=(a == 1),
                            )
                        f_t = fin_pool.tile([P, C], f32, tag="f")
                        nc.scalar.activation(
                            out=f_t, in_=ps_p, func=AF.Copy, scale=inv_t
                        )
                        nc.vector.tensor_add(f_t, f_t, xn[:, t, :])
                        nc.sync.dma_start(
                            out=out_d[t * P:(t + 1) * P, :], in_=f_t
                        )

            po0 = run_superblock(0)
            invs0 = early_epilogue(0, po0)
            po1 = run_superblock(1)
            invs1 = early_epilogue(1, po1)
            late_epilogue(0, invs0)
            late_epilogue(1, invs1)

    nc.compile()
    return nc


def _get_nc():
    if "nc" not in _CACHE:
        _CACHE["nc"] = _build_bass()
    return _CACHE["nc"]


def make_in_maps(inputs):
    x = np.ascontiguousarray(np.asarray(inputs["inputs"], np.float32)).reshape(4, NK, C)
    ctx = np.ascontiguousarray(np.asarray(inputs["context"], np.float32)).reshape(4, NK, C)
    shared = {
        k: np.ascontiguousarray(np.asarray(inputs[k], np.float32))
        for k in ("Wq", "Wk", "Wv", "Wp", "bq", "bk", "bv", "bp", "gamma", "beta")
    }
    ctxT_b = [np.ascontiguousarray(ctx[b].T) for b in range(4)]
    in_maps = []
    for core in range(8):
        b, h = divmod(core, 2)
        m = dict(shared)
        m["x"] = np.ascontiguousarray(x[b, h * NQ:(h + 1) * NQ])
        m["ctxT"] = ctxT_b[b]
        in_maps.append(m)
    return in_maps


def kernel(**inputs):
    global LAST_RESULTS
    from concourse.bass_utils import run_bass_kernel_spmd

    nc = _get_nc()
    in_maps = make_in_maps(inputs)
    res = run_bass_kernel_spmd(nc, in_maps, core_ids=list(range(8)))
    LAST_RESULTS = res
    full = np.empty((4, NK, C), np.float32)
    for core in range(8):
        b, h = divmod(core, 2)
        full[b, h * NQ:(h + 1) * NQ] = res.results[core]["out"]
    return full.reshape(4, 64, 64, 256)


# revision 18
# speedup vs baseline: 1.1274x; 1.1274x over previous
"""Cross-attention block (LN -> QKV -> full softmax attention -> proj + residual)
as a Bass/Tile kernel for 8 Trainium2 NeuronCores.

Sharding (hardcoded for B=4, H=W=64, C=U=256):
  core c handles batch b = c//2 and query-half h = c%2 (2048 of 4096 query
  positions), with K/V computed from the full 4096-position context of batch b
  (replicated inside the 2-core group). No collectives needed.

Per-core structure (P = 128 partitions, fp32r single-pass matmuls):
  ctxT [C, keys]   : transposed on the host, DMA'd straight into SBUF
  kT = Wk.T @ ctx  : [U, keys], qT = Wq.T @ x_n : [U, queries]
  v_aug            : natural [keys(P), C+2] with two ones columns (the ones
                     columns turn the attention matmul into a softmax-
                     denominator accumulator; width padded even for f32r)
  scoresT          : [keys(P), queries] PSUM -> exp on ACT (no max-sub:
                     scores are O(+-8), fp32 exp is safe)
  attention        : superblocks of 1024 queries = 2 psum blocks of 512;
                     out = v_aug.T @ p accumulates transposed [C | denom, q]
                     so the projection needs no extra transpose; each kT / v
                     weight load serves 2 matmuls so LDWEIGHTS stays hidden
  epilogue         : denom row -> reciprocal -> tiny PE transpose to [q, 1],
                     proj = atT.T @ Wp scaled by 1/denom on the copy-out,
                     + (x_n + bp) residual
"""

import numpy as np

P = 128
C = 256
U = 256
NQ = 2048          # queries per core
NK = 4096          # keys per core
QT = NQ // P       # 16 query tiles
KT = NK // P       # 32 key tiles
IB = 512           # psum block width (queries)
SB = 1024          # superblock: 2 psum blocks share each kT / v weight load
NSB = NQ // SB     # 2
SCALE = float(U) ** -0.5
LN_EPS = 1e-3

_CACHE = {}
LAST_RESULTS = None


def _build_bass():
    import concourse.bass as bass
    import concourse.tile as tile
    from concourse import bacc, mybir
    from concourse.masks import make_identity

    f32 = mybir.dt.float32
    f32r = mybir.dt.float32r
    bf16 = mybir.dt.bfloat16
    AF = mybir.ActivationFunctionType

    nc = bacc.Bacc("TRN2", debug=False, num_devices=8)

    x_d = nc.dram_tensor("x", [NQ, C], f32, kind="ExternalInput").ap()
    # f32r so the straight bit-copy DMA into the f32r SBUF slab is not a cast
    ctxT_d = nc.dram_tensor("ctxT", [C, NK], f32r, kind="ExternalInput").ap()
    w_d = {
        name: nc.dram_tensor(name, [C, U], f32, kind="ExternalInput").ap()
        for name in ("Wq", "Wk", "Wv", "Wp")
    }
    b_d = {
        name: nc.dram_tensor(name, [U], f32, kind="ExternalInput").ap()
        for name in ("bq", "bk", "bv", "bp")
    }
    gamma_d = nc.dram_tensor("gamma", [C], f32, kind="ExternalInput").ap()
    beta_d = nc.dram_tensor("beta", [C], f32, kind="ExternalInput").ap()
    out_d = nc.dram_tensor("out", [NQ, C], f32, kind="ExternalOutput").ap()

    def bcast(ap1d):
        # [N] dram vector -> [P, N] broadcast read (partition step 0)
        return bass.AP(tensor=ap1d.tensor, offset=ap1d.offset, ap=[[0, P], *ap1d.ap])

    with tile.TileContext(nc) as tc:
        from contextlib import ExitStack

        with ExitStack() as es:
            singles = es.enter_context(tc.tile_pool(name="singles", bufs=1))
            psum = es.enter_context(tc.tile_pool(name="psum", bufs=2, space="PSUM"))
            work = es.enter_context(tc.tile_pool(name="work", bufs=4))
            ln = es.enter_context(tc.tile_pool(name="ln", bufs=4))
            p_pool = es.enter_context(tc.tile_pool(name="p_pool", bufs=4))
            inv_pool = es.enter_context(tc.tile_pool(name="inv_pool", bufs=4))
            fin_pool = es.enter_context(tc.tile_pool(name="fin_pool", bufs=4))

            # ---- constants ----
            ident = singles.tile([P, P], f32)
            make_identity(nc, ident)
            eps_t = singles.tile([P, 1], f32)
            nc.vector.memset(eps_t, LN_EPS)
            one11 = singles.tile([1, 1], f32)
            nc.vector.memset(one11, 1.0)

            # DMA order on the sync queue is the startup critical path:
            # Wk + biases first (kT needs them), then ctxT chunks interleaved
            # with the x tiles (LN runs on DVE/ACT while PE does kT/v).
            w_stage = {}
            w_sb = {}

            def dma_w(name):
                t0 = work.tile([P, 2, U], f32, tag="wstage", name=f"sb0_{name}", bufs=2)
                nc.sync.dma_start(out=t0, in_=w_d[name].rearrange("(a p) u -> p a u", p=P))
                w_stage[name] = t0

            def cast_w(name):
                # round to f32r once so the PE single-pass matmul can consume it
                t = singles.tile([P, 2, U], f32r, name=f"sb_{name}")
                nc.vector.tensor_copy(out=t, in_=w_stage[name])
                w_sb[name] = t

            dma_w("Wk")
            dma_w("Wv")
            bk_t = singles.tile([P, 2], f32)
            nc.sync.dma_start(out=bk_t, in_=b_d["bk"].rearrange("(a p) -> p a", p=P))
            bq_t = singles.tile([P, 2], f32)
            nc.sync.dma_start(out=bq_t, in_=b_d["bq"].rearrange("(a p) -> p a", p=P))
            cast_w("Wk")
            cast_w("Wv")
            # f32r all-ones column pair, for the denominator partition-reduce
            ones_t = singles.tile([P, 2], f32r)

            nc.scalar.activation(
                out=ones_t, in_=bk_t, func=AF.Copy, scale=0.0, bias=1.0
            )
            bv_b = singles.tile([P, C], f32)
            nc.gpsimd.dma_start(out=bv_b, in_=bcast(b_d["bv"]))
            bp_b = singles.tile([P, C], f32)
            nc.gpsimd.dma_start(out=bp_b, in_=bcast(b_d["bp"]))
            gamma_b = singles.tile([P, C], f32)
            nc.gpsimd.dma_start(out=gamma_b, in_=bcast(gamma_d))
            beta_b = singles.tile([P, C], f32)
            nc.gpsimd.dma_start(out=beta_b, in_=bcast(beta_d))

            # ---- persistent slabs ----
            xn = singles.tile([P, QT, C], f32)         # x_n natural (+bp later)
            xnT = singles.tile([P, 2, NQ], f32r)       # x_n transposed [C, rows]
            kT = singles.tile([P, 2, NK], f32r)        # k transposed [U, keys]
            qT = singles.tile([P, 2, NQ], f32r)        # q transposed [U, queries]
            v_aug = singles.tile([P, KT, C + 2], f32r)  # v natural + ones cols
            atT = singles.tile([P, 2, NQ], f32r)       # attn-out unnormalized [C, q]

            # ctxT arrives pre-transposed from the host; chunked DMA (with the
            # x tiles interleaved) so kT matmuls start on chunk 0 while the
            # rest streams in and LN has inputs early. Its pool is released
            # after the v loop so the attention pools can reuse the space.
            ctxp = tc.alloc_tile_pool(name="ctxp", bufs=1)
            ctxT = ctxp.tile([P, 2, NK], f32r)      # context transposed [C, keys]
            ctxT_src = ctxT_d.rearrange("(a p) j -> p a j", p=P)
            x_t3 = x_d.rearrange("(t p) c -> t p c", p=P)
            x_tiles = []
            NCH = 8
            CHW = NK // NCH
            for ch in range(NCH):
                nc.sync.dma_start(
                    out=ctxT[:, :, ch * CHW:(ch + 1) * CHW],
                    in_=ctxT_src[:, :, ch * CHW:(ch + 1) * CHW],
                )
                for t in range(ch * 2, ch * 2 + 2):
                    x_t = work.tile([P, C], f32, tag="x", name=f"x_{t}", bufs=8)
                    nc.sync.dma_start(out=x_t, in_=x_t3[t])
                    x_tiles.append(x_t)

            # ---- kT[u, j] = sum_c Wk[c, u] * ctx[j, c] (first PE work) ----
            for n in range(NK // 512):
                for b2 in range(2):
                    ps = psum.tile([P, 512], f32, tag="o", bufs=6, name="ps_k")
                    for a in range(2):
                        nc.tensor.matmul(
                            ps,
                            lhsT=w_sb["Wk"][:, a, b2 * P:(b2 + 1) * P],
                            rhs=ctxT[:, a, n * 512:(n + 1) * 512],
                            start=(a == 0),
                            stop=(a == 1),
                        )
                    nc.scalar.activation(
                        out=kT[:, b2, n * 512:(n + 1) * 512],
                        in_=ps,
                        func=AF.Identity,
                        bias=bk_t[:, b2:b2 + 1],
                    )

            dma_w("Wq")
            dma_w("Wp")

            def emit_ln(t):
                # layernorm of x tile t on DVE/ACT -> xn slab
                x_t = x_tiles[t]
                st = ln.tile([P, 6], f32, tag="st")
                nc.vector.bn_stats(out=st, in_=x_t)
                mv = ln.tile([P, 2], f32, tag="mv")
                nc.vector.bn_aggr(out=mv, in_=st)
                rstd = ln.tile([P, 1], f32, tag="rstd")
                nc.scalar.activation(out=rstd, in_=mv[:, 1:2], func=AF.Sqrt, bias=eps_t)
                nc.vector.reciprocal(rstd, rstd)
                nmr = ln.tile([P, 1], f32, tag="nmr")
                nc.vector.tensor_mul(nmr, mv[:, 0:1], rstd)
                nc.vector.tensor_scalar_mul(nmr, nmr, -1.0)
                # x_n = x * rstd - mu * rstd, then gamma/beta
                nc.scalar.activation(
                    out=xn[:, t, :], in_=x_t, func=AF.Identity, bias=nmr, scale=rstd
                )
                nc.vector.tensor_mul(xn[:, t, :], xn[:, t, :], gamma_b)
                nc.vector.tensor_add(xn[:, t, :], xn[:, t, :], beta_b)

            # ---- v natural [j, c] (+ ones cols), LN interleaved so the DVE
            # alternates between v bias-adds (PE-paced) and LN (DMA-paced) ----
            for t in range(KT):
                ps = psum.tile([P, C], f32, tag="o", bufs=6, name="ps_v")
                for a in range(2):
                    nc.tensor.matmul(
                        ps,
                        lhsT=ctxT[:, a, t * P:(t + 1) * P],
                        rhs=w_sb["Wv"][:, a, :],
                        start=(a == 0),
                        stop=(a == 1),
                    )
                nc.vector.tensor_add(v_aug[:, t, 0:C], ps, bv_b)
                nc.scalar.activation(
                    out=v_aug[:, t, C:C + 2], in_=gamma_b[:, 0:2],
                    func=AF.Copy, scale=0.0, bias=1.0,
                )
                if t % 2 == 0:
                    emit_ln(t // 2)

            ctxp.release()

            cast_w("Wq")
            cast_w("Wp")

            # ---- transpose x_n (PE; LN is long done by now) ----
            for t in range(QT):
                for a in range(2):
                    pt = psum.tile([P, P], f32, tag="s", name="pt_xn")
                    nc.tensor.transpose(pt, xn[:, t, a * P:(a + 1) * P], ident)
                    nc.vector.tensor_copy(out=xnT[:, a, t * P:(t + 1) * P], in_=pt)
                # residual base = x_n + bp (after the transpose reads x_n)
                nc.vector.tensor_add(xn[:, t, :], xn[:, t, :], bp_b)

            # ---- qT[u, i] = sum_c Wq[c, u] * x_n[i, c] ----
            for n in range(NQ // 512):
                for b2 in range(2):
                    ps = psum.tile([P, 512], f32, tag="o", bufs=6, name="ps_q")
                    for a in range(2):
                        nc.tensor.matmul(
                            ps,
                            lhsT=w_sb["Wq"][:, a, b2 * P:(b2 + 1) * P],
                            rhs=xnT[:, a, n * 512:(n + 1) * 512],
                            start=(a == 0),
                            stop=(a == 1),
                        )
                    nc.scalar.activation(
                        out=qT[:, b2, n * 512:(n + 1) * 512],
                        in_=ps,
                        func=AF.Identity,
                        bias=bq_t[:, b2:b2 + 1],
                    )

            # ---- attention: per superblock of 1024 queries ----
            # po[blk] = [atT chunk0 [128,512], atT chunk1 [128,512]] accumulated
            # over all 32 key tiles; the softmax denominator accumulates on the
            # otherwise-idle DVE (acc += p), partition-reduced by one tiny
            # matmul with a ones weight at the end of the superblock.
            VCH = ((0, P), (P, 2 * P))
            acc_pool = es.enter_context(tc.tile_pool(name="acc_pool", bufs=4))

            def run_superblock(sb):
                po = [
                    [
                        psum.tile(
                            [hi - lo, IB], f32, tag="o", bufs=6,
                            name=f"po{sb}_{blk}_{ci}",
                        )
                        for ci, (lo, hi) in enumerate(VCH)
                    ]
                    for blk in range(2)
                ]

                def emit_attn(pts, j):
                    for ci, (lo, hi) in enumerate(VCH):
                        for blk in range(2):
                            nc.tensor.matmul(
                                po[blk][ci],
                                lhsT=v_aug[:, j, lo:hi],
                                rhs=pts[blk],
                                start=(j == 0),
                                stop=(j == KT - 1),
                            )

                accs = [
                    acc_pool.tile([P, IB], f32r, tag="acc", name=f"acc{sb}_{blk}")
                    for blk in range(2)
                ]
                pend = None
                for j in range(KT):
                    pss = []
                    for blk in range(2):
                        ps = psum.tile([P, IB], f32, tag="s", name="ps_s")
                        pss.append(ps)
                    for a in range(2):
                        for blk in range(2):
                            nc.tensor.matmul(
                                pss[blk],
                                lhsT=kT[:, a, j * P:(j + 1) * P],
                                rhs=qT[:, a, sb * SB + blk * IB:sb * SB + (blk + 1) * IB],
                                start=(a == 0),
                                stop=(a == 1),
                            )
                    pts = []
                    for blk in range(2):
                        pt = p_pool.tile([P, IB], f32r, tag="p", name="p_exp")
                        nc.scalar.activation(out=pt, in_=pss[blk], func=AF.Exp, scale=SCALE)
                        # running denominator on DVE: acc[r, i] += p[r, i]
                        if j == 0:
                            nc.vector.tensor_copy(out=accs[blk], in_=pt)
                        else:
                            nc.vector.tensor_add(accs[blk], accs[blk], pt)
                        pts.append(pt)
                    # 1-deep software pipeline: previous j's attention matmuls
                    # are emitted after this j's score matmuls
                    if pend is not None:
                        emit_attn(*pend)
                    pend = (pts, j)
                emit_attn(*pend)
                return po, accs

            def early_epilogue(sb, po, accs):
                # free the po psum banks: copy atT chunks out, build 1/denom
                invs = []
                for blk in range(2):
                    qlo = sb * SB + blk * IB
                    for ci in range(2):
                        nc.vector.tensor_copy(
                            out=atT[:, ci, qlo:qlo + IB], in_=po[blk][ci]
                        )
                    # partition-reduce the running p-sums: [128,512] -> [2,512]
                    ps_d = psum.tile([2, IB], f32, tag="s", name="ps_d")
                    nc.tensor.matmul(
                        ps_d, lhsT=ones_t, rhs=accs[blk], start=True, stop=True
                    )
                    inv_row = inv_pool.tile([1, IB], f32, tag="invrow")
                    nc.vector.reciprocal(inv_row, ps_d[0:1, :])
                    invs.append(inv_row)
                return invs

            def late_epilogue(sb, invs):
                for blk in range(2):
                    inv_row = invs[blk]
                    for s in range(IB // P):
                        t = (sb * SB + blk * IB) // P + s
                        # transpose inv_row chunk [1,128] -> [128,1] via PE
                        ps_i = psum.tile([P, 1], f32, tag="s", name="ps_i")
                        nc.tensor.matmul(
                            ps_i,
                            lhsT=inv_row[0:1, s * P:(s + 1) * P],
                            rhs=one11,
                            start=True,
                            stop=True,
                        )
                        inv_t = inv_pool.tile([P, 1], f32, tag="invt", bufs=8)
                        nc.vector.tensor_copy(out=inv_t, in_=ps_i)
                        # proj, scaled by 1/denom on copy-out, + residual
                        ps_p = psum.tile([P, C], f32, tag="s", name="ps_p")
                        for a in range(2):
                            nc.tensor.matmul(
                                ps_p,
                                lhsT=atT[:, a, t * P:(t + 1) * P],
                                rhs=w_sb["Wp"][:, a, :],
                                start=(a == 0),
                                stop=(a == 1),
                            )
                        f_t = fin_pool.tile([P, C], f32, tag="f")
                        nc.scalar.activation(
                            out=f_t, in_=ps_p, func=AF.Copy, scale=inv_t
                        )
                        nc.vector.tensor_add(f_t, f_t, xn[:, t, :])
                        nc.sync.dma_start(
                            out=out_d[t * P:(t + 1) * P, :], in_=f_t
                        )

            po0, accs0 = run_superblock(0)
            invs0 = early_epilogue(0, po0, accs0)
            po1, accs1 = run_superblock(1)
            invs1 = early_epilogue(1, po1, accs1)
            late_epilogue(0, invs0)
            late_epilogue(1, invs1)

    nc.compile()
    return nc


def _get_nc():
    if "nc" not in _CACHE:
        _CACHE["nc"] = _build_bass()
    return _CACHE["nc"]


def make_in_maps(inputs):
    x = np.ascontiguousarray(np.asarray(inputs["inputs"], np.float32)).reshape(4, NK, C)
    ctx = np.ascontiguousarray(np.asarray(inputs["context"], np.float32)).reshape(4, NK, C)
    shared = {
        k: np.ascontiguousarray(np.asarray(inputs[k], np.float32))
        for k in ("Wq", "Wk", "Wv", "Wp", "bq", "bk", "bv", "bp", "gamma", "beta")
    }
    ctxT_b = [np.ascontiguousarray(ctx[b].T) for b in range(4)]
    in_maps = []
    for core in range(8):
        b, h = divmod(core, 2)
        m = dict(shared)
        m["x"] = np.ascontiguousarray(x[b, h * NQ:(h + 1) * NQ])
        m["ctxT"] = ctxT_b[b]
        in_maps.append(m)
    return in_maps


def kernel(**inputs):
    global LAST_RESULTS
    from concourse.bass_utils import run_bass_kernel_spmd

    nc = _get_nc()
    in_maps = make_in_maps(inputs)
    res = run_bass_kernel_spmd(nc, in_maps, core_ids=list(range(8)))
    LAST_RESULTS = res
    full = np.empty((4, NK, C), np.float32)
    for core in range(8):
        b, h = divmod(core, 2)
        full[b, h * NQ:(h + 1) * NQ] = res.results[core]["out"]
    return full.reshape(4, 64, 64, 256)
